# revision 31
# baseline (speedup 1.0000x reference)
"""Trainium2 Bass kernel for Exaone4-style GQA attention block (T=2048, HID=4096,
H=32 q-heads, HK=8 kv-heads, D=128, sliding window 1023, QK-RMSNorm + NeoX RoPE).

Sharding: tensor-parallel over heads across 8 NeuronCores. Core m owns q-heads
[4m, 4m+4) and kv-head m (GQA group-aligned), plus the matching o_proj column
slice; per-core partial outputs are summed on the host (the all-reduce).

Device layout notes:
 - qkv projection is computed transposed ([feature, t]) so attention works in
   the S^T = K^T.T @ Q^T layout; softmax sums over the partition axis are done
   with an all-ones [128,128] matmul whose output carries the sum replicated
   on every partition row (no row-broadcast step needed anywhere).
 - RoPE rotate-half is done with partition-base-shifted DVE ops against a
   host-rotated sin table (no SBUF-SBUF DMA, no DRAM bounce).
 - exp(S^T) is decoupled from PV: scores go PSUM -> ACT exp -> SBUF es tiles,
   and the PV/rowsum matmuls consume the es tiles afterwards, so the PE queue
   never waits on ACT latency.
 - All large matmuls use bf16 operands with fp32 PSUM accumulation.
"""

import sys

import numpy as np

if "/opt/trn_rl_repo" not in sys.path:
    sys.path.insert(0, "/opt/trn_rl_repo")

import ml_dtypes

BF16 = ml_dtypes.bfloat16

HID = 4096
H = 32
HK = 8
D = 128
WIN = 1023
THETA = 1000000.0
EPS = 1e-6
SCALE = D ** -0.5
M = 8            # cores
QH = H // M      # q heads per core (4)
NJ = QH + 2      # j-blocks in qkv^T output (4 q + 1 k + 1 v)
TB = 512         # t free-dim block
NEG = -1.0e30

_PROG_CACHE = {}


def _build_program(T):
    """Build the (single-core SPMD) Bass program for sequence length T."""
    from contextlib import ExitStack

    import concourse.bass as bass  # noqa: F401
    import concourse.tile as tile
    from concourse import bacc, mybir
    from concourse.masks import make_identity

    f32 = mybir.dt.float32
    bf = mybir.dt.bfloat16

    NT = T // TB          # number of t blocks
    NC = HID // 128       # contraction chunks
    NOB = HID // 128      # output row blocks
    NWC = 16              # w chunks (2 c-chunks each)

    nc = bacc.Bacc(
        "TRN2",
        target_bir_lowering=False,
        debug=False,
        enable_asserts=False,
        num_devices=M,
    )

    # x pre-tiled on host: block (tb, cq) = [128, 4*TB], 4 c-chunks interleaved
    # per partition row (4KB contiguous per partition per DMA)
    xT_h = nc.dram_tensor(
        "xT", [(T // TB) * (HID // 512) * 128, 4 * TB], bf, kind="ExternalInput"
    )
    # w pre-tiled on host: chunk k = [128, 8*768] contiguous per partition so
    # the DMA is 128 fat descriptors, not a 1024-descriptor gather.
    wq_h = nc.dram_tensor("wqkvT", [NWC * 128, (NC // NWC) * NJ * 128], bf,
                          kind="ExternalInput")
    # wo pre-tiled the same way: chunk jc = [128, HID] contiguous.
    wo_h = nc.dram_tensor("woT", [QH * 128, HID], bf, kind="ExternalInput")
    cwq_h = nc.dram_tensor("cwq", [128, T], bf, kind="ExternalInput")
    swq_h = nc.dram_tensor("swq", [128, T], bf, kind="ExternalInput")
    cwk_h = nc.dram_tensor("cwk", [128, T], bf, kind="ExternalInput")
    swk_h = nc.dram_tensor("swk", [128, T], bf, kind="ExternalInput")
    maskd_h = nc.dram_tensor("maskd", [128, 128], f32, kind="ExternalInput")
    maskw_h = nc.dram_tensor("maskw", [128, 128], f32, kind="ExternalInput")
    # out pre-tiled: block (tb, obq) = [128, 4*TB] (ob quads interleaved per row)
    outT_h = nc.dram_tensor(
        "outT", [(T // TB) * (HID // 512) * 128, 4 * TB], bf, kind="ExternalOutput"
    )

    xTr = xT_h.ap().rearrange("(b p) u -> b p u", p=128)
    wqr = wq_h.ap().rearrange("(k p) u -> k p u", p=128)
    wor = wo_h.ap().rearrange("(jc p) o -> jc p o", p=128)
    outr = outT_h.ap().rearrange("(b p) u -> b p u", p=128)

    mult = mybir.AluOpType.mult
    add = mybir.AluOpType.add
    Exp = mybir.ActivationFunctionType.Exp
    Sqrt = mybir.ActivationFunctionType.Sqrt

    with tile.TileContext(nc) as tc, ExitStack() as ctx:
        singles = ctx.enter_context(tc.tile_pool(name="singles", bufs=1))
        persist = ctx.enter_context(tc.tile_pool(name="persist", bufs=1))
        xpool = ctx.enter_context(tc.tile_pool(name="xpool", bufs=4))
        stpool = ctx.enter_context(tc.tile_pool(name="stpool", bufs=2))
        qTp = ctx.enter_context(tc.tile_pool(name="qTp", bufs=2))
        sqp = ctx.enter_context(tc.tile_pool(name="sqp", bufs=2))
        rmsp = ctx.enter_context(tc.tile_pool(name="rmsp", bufs=2))
        ropep = ctx.enter_context(tc.tile_pool(name="ropep", bufs=2))
        esp = ctx.enter_context(tc.tile_pool(name="esp", bufs=24))
        rbp = ctx.enter_context(tc.tile_pool(name="rbp", bufs=2))
        attnp = ctx.enter_context(tc.tile_pool(name="attnp", bufs=2))
        outp = ctx.enter_context(tc.tile_pool(name="outp", bufs=3))
        # PSUM: every tile is <= one bank; a single tag with 8 rotating slots
        # covers all 8 banks and lets phases overlap freely.
        psum = ctx.enter_context(tc.tile_pool(name="psum", bufs=8, space="PSUM"))

        # ---- resident constants -------------------------------------------
        # w chunks on the scalar queue, x tiles on sync, the rest on gpsimd:
        # three independent DMA paths so the first qkv matmul can start ~5us
        # in instead of waiting for 12MB of weights/tables to land.
        w_sb = [singles.tile([128, NC // NWC, NJ * 128], bf, name=f"w{k}")
                for k in range(NWC)]
        # first two chunks ride the gpsimd queue so they land in parallel
        # with the scalar queue pulling the rest; the first qkv matmuls only
        # need chunk 0 + the first x tile.
        for k in range(NWC):
            eng = nc.gpsimd if k < 4 else nc.scalar
            eng.dma_start(w_sb[k], wqr[k].rearrange("p (c j) -> p c j", j=NJ * 128))
        # first two wo chunks ahead of the rope tables on gpsimd: C(0) wants
        # them by ~65us, the tables aren't read until rope(0) at ~45us.
        wo_sb = [singles.tile([128, HID], bf, name=f"wo{jc}") for jc in range(QH)]
        for jc in range(2):
            nc.gpsimd.dma_start(wo_sb[jc], wor[jc])
        cwq_sb = singles.tile([128, T], bf)
        nc.gpsimd.dma_start(cwq_sb, cwq_h.ap())
        swq_sb = singles.tile([128, T], bf)
        nc.gpsimd.dma_start(swq_sb, swq_h.ap())
        cwk_sb = singles.tile([128, T], bf)
        nc.gpsimd.dma_start(cwk_sb, cwk_h.ap())
        swk_sb = singles.tile([128, T], bf)
        nc.gpsimd.dma_start(swk_sb, swk_h.ap())
        maskd_sb = singles.tile([128, 128], f32)
        nc.gpsimd.dma_start(maskd_sb, maskd_h.ap())
        maskw_sb = singles.tile([128, 128], f32)
        nc.gpsimd.dma_start(maskw_sb, maskw_h.ap())
        ident = singles.tile([128, 128], bf)
        make_identity(nc, ident)
        ones128 = singles.tile([128, 128], bf)
        nc.vector.memset(ones128, 1.0)
        eps_sb = singles.tile([128, 1], f32)
        nc.vector.memset(eps_sb, EPS)

        # ---- persistent activations ---------------------------------------
        kT = persist.tile([128, T], bf)              # rope'd+normed k^T
        Vt = persist.tile([128, T // 128, 128], bf)  # v in [s, d] layout

        qTs = {}

        def phase_a(tb):
            """qkv projection + rmsnorm + rope for t block tb."""
            t0 = tb * TB
            ts_ = slice(t0, t0 + TB)
            stage = stpool.tile([128, NJ, TB], bf, tag="stage", name=f"stage_{tb}")
            qTt = qTp.tile([128, QH, TB], bf, tag="qT", name=f"qT_{tb}")
            qTs[tb] = qTt
            # tb 0 runs before any attention needs PSUM, so it can hold all
            # six banks and read x once — halves the startup HBM burst.
            groups = [range(NJ)] if tb == 0 else [range(3), range(3, NJ)]
            for js in groups:
                ps_g = [
                    psum.tile([128, TB], f32, name=f"psqkv_{tb}_{j}", tag="bank")
                    for j in js
                ]
                for cq in range(NC // 4):
                    xc = xpool.tile([128, 4, TB], bf, tag="xc",
                                    name=f"xc_{tb}_{js[0]}_{cq}")
                    nc.sync.dma_start(
                        xc,
                        xTr[tb * (NC // 4) + cq].rearrange("p (ci u) -> p ci u", u=TB),
                    )
                    for ci in range(4):
                        c = 4 * cq + ci
                        for ji, j in enumerate(js):
                            nc.tensor.matmul(
                                ps_g[ji],
                                lhsT=w_sb[c // 2][:, c % 2, j * 128 : (j + 1) * 128],
                                rhs=xc[:, ci, :],
                                start=(c == 0),
                                stop=(c == NC - 1),
                            )
                for ji, j in enumerate(js):
                    nc.any.tensor_copy(stage[:, j], ps_g[ji])

            # v: transpose [d, t] -> [s, d] blocks via PE
            for u in range(TB // 128):
                ps_t = psum.tile([128, 128], bf, name=f"pst_{tb}_{u}", tag="bank")
                nc.tensor.transpose(ps_t, stage[:, QH + 1, u * 128 : (u + 1) * 128], ident)
                nc.any.tensor_copy(Vt[:, tb * (TB // 128) + u, :], ps_t)

            # rms scale + rope per j-block. The ones128 matmul leaves
            # sum_d(x^2) replicated on every output row, so the whole scale
            # chain stays [128, TB] and no row-broadcast is needed.
            for j in range(QH + 1):
                sq = sqp.tile([128, TB], bf, tag="sq", name=f"sq_{tb}_{j}")
                nc.vector.tensor_tensor(sq, stage[:, j], stage[:, j], mult)
                ps_ss = psum.tile([128, TB], f32, name=f"psss_{tb}_{j}", tag="bank")
                nc.tensor.matmul(ps_ss, lhsT=ones128, rhs=sq, start=True, stop=True)
                rms = rmsp.tile([128, TB], f32, tag="rms", name=f"rms_{tb}_{j}")
                nc.scalar.activation(rms, ps_ss, Sqrt, bias=eps_sb, scale=1.0 / D)
                scl = rmsp.tile([128, TB], f32, tag="scl", name=f"scl_{tb}_{j}")
                nc.vector.reciprocal_approx_fast(scl, rms)

                cw = cwq_sb if j < QH else cwk_sb
                sw = swq_sb if j < QH else swk_sb
                # rope on the UNnormalized row (rotation commutes with the
                # per-column scale); rotate-half via output-partition-shifted
                # DVE ops against the host-rotated sin table.
                a_t = ropep.tile([128, TB], f32, tag="a_t", name=f"at_{tb}_{j}")
                nc.vector.tensor_tensor(a_t, stage[:, j], cw[:, ts_], mult)
                b_t = ropep.tile([128, TB], f32, tag="b_t", name=f"bt_{tb}_{j}")
                nc.vector.tensor_tensor(
                    b_t[0:64, :], stage[64:128, j], sw[64:128, ts_], mult
                )
                nc.vector.tensor_tensor(
                    b_t[64:128, :], stage[0:64, j], sw[0:64, ts_], mult
                )
                nc.vector.tensor_tensor(a_t, a_t, b_t, add)
                dest = qTt[:, j] if j < QH else kT[:, ts_]
                nc.vector.tensor_tensor(dest, a_t, scl, mult)

        attnTs = {}

        def phase_b(tb):
            """attention for t block tb (attnT kept for phase_c)."""
            t0 = tb * TB
            # o = sb - 4*tb; o=0 (full col range) goes FIRST so the
            # start=True PV/rowsum matmuls cover the whole bank; later
            # partial-range matmuls accumulate onto uniformly-written bytes
            # (CoreSim requires this; matches HW has_written semantics).
            obs = [0] + [o for o in range(-8, 4) if o != 0 and 4 * tb + o >= 0]

            def cr(o):
                if o >= 0:
                    return 128 * o, TB
                elif o >= -4:
                    return 0, TB
                else:
                    return 0, 128 * (o + 9)

            attnT = attnp.tile([128, QH, TB], bf, tag="attnT", name=f"attnT_{tb}")
            attnTs[tb] = attnT
            for hp in range(QH // 2):
                heads = (2 * hp, 2 * hp + 1)
                es = {}
                nobs = len(obs)
                rs_ps = {
                    h: psum.tile([128, TB], f32, name=f"psr_{tb}_{h}", tag="bank")
                    for h in heads
                }

                def rs_mm(h, oi):
                    # rowsum via ones128: result replicated on all rows, so
                    # the reciprocal is directly usable as a [128, TB] factor.
                    c0, c1 = cr(obs[oi])
                    nc.tensor.matmul(
                        rs_ps[h][:, c0:c1],
                        lhsT=ones128,
                        rhs=es[h, oi][:, c0:c1],
                        start=(oi == 0),
                        stop=(oi == nobs - 1),
                        skip_group_check=True,
                    )

                # score + exp pass, with the rowsum matmuls chasing the exp
                # stream at a fixed lag so they never queue up behind a full
                # head's worth of ACT latency.
                LAG = 3
                for oi, o in enumerate(obs):
                    sb = 4 * tb + o
                    c0, c1 = cr(o)
                    for h in heads:
                        ps_s = psum.tile(
                            [128, TB], f32, name=f"pss_{tb}_{h}_{oi}", tag="bank"
                        )
                        nc.tensor.matmul(
                            ps_s[:, c0:c1],
                            lhsT=kT[:, sb * 128 : (sb + 1) * 128],
                            rhs=qTs[tb][:, h, c0:c1],
                            start=True,
                            stop=True,
                        )
                        if o >= 0:  # causal strip at cols [128o, 128o+128)
                            u0 = 128 * o
                            nc.vector.tensor_tensor(
                                ps_s[:, u0 : u0 + 128], ps_s[:, u0 : u0 + 128],
                                maskd_sb, add,
                            )
                        elif o <= -5:  # window strip
                            u0 = 128 * (o + 8)
                            nc.vector.tensor_tensor(
                                ps_s[:, u0 : u0 + 128], ps_s[:, u0 : u0 + 128],
                                maskw_sb, add,
                            )
                        e = esp.tile([128, TB], bf, tag="es", name=f"es_{tb}_{h}_{oi}")
                        nc.scalar.activation(e[:, c0:c1], ps_s[:, c0:c1], Exp)
                        es[h, oi] = e
                    if oi >= LAG:
                        for h in heads:
                            rs_mm(h, oi - LAG)
                for oi in range(max(0, nobs - LAG), nobs):
                    for h in heads:
                        rs_mm(h, oi)
                for h in heads:
                    rb = rbp.tile([128, TB], f32, tag="rb", name=f"rb_{tb}_{h}")
                    nc.vector.reciprocal_approx_fast(rb, rs_ps[h])
                    pv = psum.tile([128, TB], f32, name=f"pspv_{tb}_{h}", tag="bank")
                    for oi, o in enumerate(obs):
                        sb = 4 * tb + o
                        c0, c1 = cr(o)
                        nc.tensor.matmul(
                            pv[:, c0:c1],
                            lhsT=Vt[:, sb, :],
                            rhs=es[h, oi][:, c0:c1],
                            start=(oi == 0),
                            stop=(oi == nobs - 1),
                            skip_group_check=True,
                        )
                    nc.vector.tensor_tensor(attnT[:, h], pv, rb, mult)

        def phase_c(tb):
            """o_proj partial for t block tb (store ob quads as one DMA)."""
            attnT = attnTs.pop(tb)
            for obp in range(NOB // 4):
                o_st = outp.tile(
                    [128, 4, TB], bf, tag="o_st", name=f"ost_{tb}_{obp}"
                )
                for oi in range(4):
                    ob = 4 * obp + oi
                    ps_o = psum.tile([128, TB], f32, name=f"pso_{tb}_{ob}", tag="bank")
                    for jc in range(QH):
                        nc.tensor.matmul(
                            ps_o,
                            lhsT=wo_sb[jc][:, ob * 128 : (ob + 1) * 128],
                            rhs=attnT[:, jc, :],
                            start=(jc == 0),
                            stop=(jc == QH - 1),
                        )
                    nc.vector.tensor_copy(o_st[:, oi, :], ps_o)
                st_eng = nc.gpsimd
                if tb == NT - 1 and obp % 2 == 1:
                    st_eng = nc.scalar
                st_eng.dma_start(
                    outr[tb * (NOB // 4) + obp].rearrange("p (oi u) -> p oi u", u=TB),
                    o_st,
                )

        # Software pipeline: phase A one t-block ahead (next block's qkv runs
        # while this block's rope/softmax chains sit on DVE/ACT), phase C one
        # block behind (o_proj matmuls fill any attention-phase PE bubbles).
        phase_a(0)
        # wo rides the gpsimd queue behind the startup w chunks/tables; the
        # gpsimd engine is idle then, so the issues aren't trapped behind
        # paced x-tile slot waits (sync) or the ACT exp bursts (scalar).
        for jc in range(2, QH):
            nc.scalar.dma_start(wo_sb[jc], wor[jc])
        phase_a(1)
        phase_b(0)
        # per iteration: C first (dense, dependency-free filler), then B so
        # its score matmuls feed the ACT exp stream before the PE runs dry,
        # then A as the low-priority filler for B's rowsum/PV exp-chasing.
        for tb in range(1, NT):
            phase_c(tb - 1)
            phase_b(tb)
            if tb + 1 < NT:
                phase_a(tb + 1)
        phase_c(NT - 1)

    nc.compile()
    return nc


def _get_program(T):
    if T not in _PROG_CACHE:
        _PROG_CACHE[T] = _build_program(T)
    return _PROG_CACHE[T]


def _host_prep(positions, hidden_states, wqkv, wo, q_norm_w, k_norm_w):
    """Build the 8 per-core input maps (host-side sharding + table prep)."""
    T = hidden_states.shape[0]
    pos = np.asarray(positions).astype(np.float64)
    hs = np.asarray(hidden_states, dtype=np.float32)
    wqkv = np.asarray(wqkv, dtype=np.float32)
    wo = np.asarray(wo, dtype=np.float32)
    qw = np.asarray(q_norm_w, dtype=np.float64)
    kw = np.asarray(k_norm_w, dtype=np.float64)

    half = D // 2
    inv_freq = 1.0 / (THETA ** (np.arange(0, D, 2, dtype=np.float64) / D))  # [64]
    th = pos[:, None] * inv_freq[None, :]          # [T, 64]
    cos = np.cos(th).T                             # [64, T] float64
    sin = np.sin(th).T

    def tables(w, scale):
        cw = np.empty((D, T), np.float64)
        sw = np.empty((D, T), np.float64)
        cw[:half] = cos * (w[:half, None] * scale)
        cw[half:] = cos * (w[half:, None] * scale)
        # sin table pre-rotated by half so the kernel's shifted-output DVE
        # ops read coefficient and source from the SAME partition range:
        #   out[0:64]  = x[64:128] * sw[64:128]  (= -sin * w_hi * x_hi)
        #   out[64:128] = x[0:64]  * sw[0:64]    (= +sin * w_lo * x_lo)
        sw[:half] = sin * (w[:half, None] * scale)
        sw[half:] = -sin * (w[half:, None] * scale)
        return cw.astype(BF16), sw.astype(BF16)

    cwq, swq = tables(qw, SCALE)
    cwk, swk = tables(kw, 1.0)

    si = np.arange(128)[:, None]
    ui = np.arange(128)[None, :]
    maskd = np.where(ui >= si, 0.0, NEG).astype(np.float32)
    maskw = np.where(ui < si, 0.0, NEG).astype(np.float32)

    # tiled layout: block (tb, cq) = [128, 4*TB]; row p holds c-chunks
    # 4cq..4cq+3 back to back (4KB contiguous per partition)
    NTb, NCq = T // TB, HID // 512
    xT = np.ascontiguousarray(
        hs.T.reshape(NCq, 4, 128, NTb, TB)
        .transpose(3, 0, 2, 1, 4)
        .reshape(NTb * NCq * 128, 4 * TB)
    ).astype(BF16)

    NWC = 16
    in_maps = []
    for m in range(M):
        wq_m = wqkv[m * QH * D : (m + 1) * QH * D]            # [512, HID]
        wk_m = wqkv[H * D + m * D : H * D + (m + 1) * D]      # [128, HID]
        wv_m = wqkv[(H + HK) * D + m * D : (H + HK) * D + (m + 1) * D]
        wqkvT_m = np.concatenate([wq_m, wk_m, wv_m], axis=0).T  # [HID, 768]
        # pre-tile: chunk k = c-chunks [2k, 2k+2) as [128, 2*768] with the
        # per-partition row contiguous (c-major within the row)
        wqkvT_m = np.ascontiguousarray(
            wqkvT_m.reshape(NWC, 2, 128, NJ * D)
            .transpose(0, 2, 1, 3)
            .reshape(NWC * 128, 2 * NJ * D)
        ).astype(BF16)
        # pre-tile wo: chunk jc = [128, HID] contiguous
        woT_m = np.ascontiguousarray(
            wo[:, m * QH * D : (m + 1) * QH * D].T.reshape(QH * 128, HID)
        ).astype(BF16)                                        # [512, HID]
        in_maps.append(
            {
                "xT": xT,
                "wqkvT": wqkvT_m,
                "woT": woT_m,
                "cwq": cwq,
                "swq": swq,
                "cwk": cwk,
                "swk": swk,
                "maskd": maskd,
                "maskw": maskw,
            }
        )
    return in_maps


def _run(in_maps, T, trace=False):
    from concourse import bass_utils

    nc = _get_program(T)
    res = bass_utils.run_bass_kernel_spmd(
        nc, in_maps, core_ids=list(range(M)), trace=trace
    )
    return res


def kernel(positions, hidden_states, wqkv, wo, q_norm_w, k_norm_w, _trace=False):
    T = hidden_states.shape[0]
    in_maps = _host_prep(positions, hidden_states, wqkv, wo, q_norm_w, k_norm_w)
    res = _run(in_maps, T, trace=_trace)
    NTb, NOBq = T // TB, HID // 512
    acc = np.zeros((NTb, NOBq, 128, 4, TB), np.float64)
    for r in res.results:
        acc += r["outT"].astype(np.float64).reshape(NTb, NOBq, 128, 4, TB)
    # untile: out[t, o] with o = (4*obq + oi)*128 + p, t = tb*TB + u
    out = np.ascontiguousarray(
        acc.transpose(0, 4, 1, 3, 2).reshape(T, HID)
    ).astype(np.float32)
    kernel._last_results = res
    return out


# revision 32
# speedup vs baseline: 1.0012x; 1.0012x over previous
"""Trainium2 Bass kernel for Exaone4-style GQA attention block (T=2048, HID=4096,
H=32 q-heads, HK=8 kv-heads, D=128, sliding window 1023, QK-RMSNorm + NeoX RoPE).

Sharding: tensor-parallel over heads across 8 NeuronCores. Core m owns q-heads
[4m, 4m+4) and kv-head m (GQA group-aligned), plus the matching o_proj column
slice; per-core partial outputs are summed on the host (the all-reduce).

Device layout notes:
 - qkv projection is computed transposed ([feature, t]) so attention works in
   the S^T = K^T.T @ Q^T layout; softmax sums over the partition axis are done
   with an all-ones [128,128] matmul whose output carries the sum replicated
   on every partition row (no row-broadcast step needed anywhere).
 - RoPE rotate-half is done with partition-base-shifted DVE ops against a
   host-rotated sin table (no SBUF-SBUF DMA, no DRAM bounce).
 - exp(S^T) is decoupled from PV: scores go PSUM -> ACT exp -> SBUF es tiles,
   and the PV/rowsum matmuls consume the es tiles afterwards, so the PE queue
   never waits on ACT latency.
 - All large matmuls use bf16 operands with fp32 PSUM accumulation.
"""

import sys

import numpy as np

if "/opt/trn_rl_repo" not in sys.path:
    sys.path.insert(0, "/opt/trn_rl_repo")

import ml_dtypes

BF16 = ml_dtypes.bfloat16

HID = 4096
H = 32
HK = 8
D = 128
WIN = 1023
THETA = 1000000.0
EPS = 1e-6
SCALE = D ** -0.5
M = 8            # cores
QH = H // M      # q heads per core (4)
NJ = QH + 2      # j-blocks in qkv^T output (4 q + 1 k + 1 v)
TB = 512         # t free-dim block
NEG = -1.0e30

_PROG_CACHE = {}


def _build_program(T):
    """Build the (single-core SPMD) Bass program for sequence length T."""
    from contextlib import ExitStack

    import concourse.bass as bass  # noqa: F401
    import concourse.tile as tile
    from concourse import bacc, mybir
    from concourse.masks import make_identity

    f32 = mybir.dt.float32
    bf = mybir.dt.bfloat16

    NT = T // TB          # number of t blocks
    NC = HID // 128       # contraction chunks
    NOB = HID // 128      # output row blocks
    NWC = 16              # w chunks (2 c-chunks each)

    nc = bacc.Bacc(
        "TRN2",
        target_bir_lowering=False,
        debug=False,
        enable_asserts=False,
        num_devices=M,
    )

    # x pre-tiled on host: block (tb, cq) = [128, 4*TB], 4 c-chunks interleaved
    # per partition row (4KB contiguous per partition per DMA)
    xT_h = nc.dram_tensor(
        "xT", [(T // TB) * (HID // 512) * 128, 4 * TB], bf, kind="ExternalInput"
    )
    # w pre-tiled on host: chunk k = [128, 8*768] contiguous per partition so
    # the DMA is 128 fat descriptors, not a 1024-descriptor gather.
    wq_h = nc.dram_tensor("wqkvT", [NWC * 128, (NC // NWC) * NJ * 128], bf,
                          kind="ExternalInput")
    # wo pre-tiled the same way: chunk jc = [128, HID] contiguous.
    wo_h = nc.dram_tensor("woT", [QH * 128, HID], bf, kind="ExternalInput")
    cwq_h = nc.dram_tensor("cwq", [128, T], bf, kind="ExternalInput")
    swq_h = nc.dram_tensor("swq", [128, T], bf, kind="ExternalInput")
    cwk_h = nc.dram_tensor("cwk", [128, T], bf, kind="ExternalInput")
    swk_h = nc.dram_tensor("swk", [128, T], bf, kind="ExternalInput")
    maskd_h = nc.dram_tensor("maskd", [128, 128], f32, kind="ExternalInput")
    maskw_h = nc.dram_tensor("maskw", [128, 128], f32, kind="ExternalInput")
    # out pre-tiled: block (tb, obq) = [128, 4*TB] (ob quads interleaved per row)
    outT_h = nc.dram_tensor(
        "outT", [(T // TB) * (HID // 512) * 128, 4 * TB], bf, kind="ExternalOutput"
    )

    xTr = xT_h.ap().rearrange("(b p) u -> b p u", p=128)
    wqr = wq_h.ap().rearrange("(k p) u -> k p u", p=128)
    wor = wo_h.ap().rearrange("(jc p) o -> jc p o", p=128)
    outr = outT_h.ap().rearrange("(b p) u -> b p u", p=128)

    mult = mybir.AluOpType.mult
    add = mybir.AluOpType.add
    Exp = mybir.ActivationFunctionType.Exp
    Sqrt = mybir.ActivationFunctionType.Sqrt

    with tile.TileContext(nc) as tc, ExitStack() as ctx:
        singles = ctx.enter_context(tc.tile_pool(name="singles", bufs=1))
        persist = ctx.enter_context(tc.tile_pool(name="persist", bufs=1))
        xpool = ctx.enter_context(tc.tile_pool(name="xpool", bufs=4))
        stpool = ctx.enter_context(tc.tile_pool(name="stpool", bufs=2))
        qTp = ctx.enter_context(tc.tile_pool(name="qTp", bufs=2))
        sqp = ctx.enter_context(tc.tile_pool(name="sqp", bufs=2))
        rmsp = ctx.enter_context(tc.tile_pool(name="rmsp", bufs=2))
        ropep = ctx.enter_context(tc.tile_pool(name="ropep", bufs=2))
        esp = ctx.enter_context(tc.tile_pool(name="esp", bufs=24))
        rbp = ctx.enter_context(tc.tile_pool(name="rbp", bufs=2))
        attnp = ctx.enter_context(tc.tile_pool(name="attnp", bufs=2))
        outp = ctx.enter_context(tc.tile_pool(name="outp", bufs=3))
        # PSUM: every tile is <= one bank; a single tag with 8 rotating slots
        # covers all 8 banks and lets phases overlap freely.
        psum = ctx.enter_context(tc.tile_pool(name="psum", bufs=8, space="PSUM"))

        # ---- resident constants -------------------------------------------
        # w chunks on the scalar queue, x tiles on sync, the rest on gpsimd:
        # three independent DMA paths so the first qkv matmul can start ~5us
        # in instead of waiting for 12MB of weights/tables to land.
        w_sb = [singles.tile([128, NC // NWC, NJ * 128], bf, name=f"w{k}")
                for k in range(NWC)]
        # first two chunks ride the gpsimd queue so they land in parallel
        # with the scalar queue pulling the rest; the first qkv matmuls only
        # need chunk 0 + the first x tile.
        for k in range(NWC):
            eng = nc.gpsimd if k < 2 else nc.scalar
            eng.dma_start(w_sb[k], wqr[k].rearrange("p (c j) -> p c j", j=NJ * 128))
        cwq_sb = singles.tile([128, T], bf)
        nc.gpsimd.dma_start(cwq_sb, cwq_h.ap())
        swq_sb = singles.tile([128, T], bf)
        nc.gpsimd.dma_start(swq_sb, swq_h.ap())
        cwk_sb = singles.tile([128, T], bf)
        nc.gpsimd.dma_start(cwk_sb, cwk_h.ap())
        swk_sb = singles.tile([128, T], bf)
        nc.gpsimd.dma_start(swk_sb, swk_h.ap())
        maskd_sb = singles.tile([128, 128], f32)
        nc.gpsimd.dma_start(maskd_sb, maskd_h.ap())
        maskw_sb = singles.tile([128, 128], f32)
        nc.gpsimd.dma_start(maskw_sb, maskw_h.ap())
        # wo tiles: chunks 0/1 ride gpsimd (emitted after phase_a(0), behind
        # the tables), chunks 2/3 ride the scalar queue behind the w chunks.
        wo_sb = [singles.tile([128, HID], bf, name=f"wo{jc}") for jc in range(QH)]
        ident = singles.tile([128, 128], bf)
        make_identity(nc, ident)
        ones128 = singles.tile([128, 128], bf)
        nc.vector.memset(ones128, 1.0)
        eps_sb = singles.tile([128, 1], f32)
        nc.vector.memset(eps_sb, EPS)

        # ---- persistent activations ---------------------------------------
        kT = persist.tile([128, T], bf)              # rope'd+normed k^T
        Vt = persist.tile([128, T // 128, 128], bf)  # v in [s, d] layout

        qTs = {}

        def phase_a(tb):
            """qkv projection + rmsnorm + rope for t block tb."""
            t0 = tb * TB
            ts_ = slice(t0, t0 + TB)
            stage = stpool.tile([128, NJ, TB], bf, tag="stage", name=f"stage_{tb}")
            qTt = qTp.tile([128, QH, TB], bf, tag="qT", name=f"qT_{tb}")
            qTs[tb] = qTt
            # tb 0 runs before any attention needs PSUM, so it can hold all
            # six banks and read x once — halves the startup HBM burst.
            groups = [range(NJ)] if tb == 0 else [range(3), range(3, NJ)]
            for js in groups:
                ps_g = [
                    psum.tile([128, TB], f32, name=f"psqkv_{tb}_{j}", tag="bank")
                    for j in js
                ]
                for cq in range(NC // 4):
                    xc = xpool.tile([128, 4, TB], bf, tag="xc",
                                    name=f"xc_{tb}_{js[0]}_{cq}")
                    nc.sync.dma_start(
                        xc,
                        xTr[tb * (NC // 4) + cq].rearrange("p (ci u) -> p ci u", u=TB),
                    )
                    for ci in range(4):
                        c = 4 * cq + ci
                        for ji, j in enumerate(js):
                            nc.tensor.matmul(
                                ps_g[ji],
                                lhsT=w_sb[c // 2][:, c % 2, j * 128 : (j + 1) * 128],
                                rhs=xc[:, ci, :],
                                start=(c == 0),
                                stop=(c == NC - 1),
                            )
                for ji, j in enumerate(js):
                    nc.any.tensor_copy(stage[:, j], ps_g[ji])

            # v: transpose [d, t] -> [s, d] blocks via PE
            for u in range(TB // 128):
                ps_t = psum.tile([128, 128], bf, name=f"pst_{tb}_{u}", tag="bank")
                nc.tensor.transpose(ps_t, stage[:, QH + 1, u * 128 : (u + 1) * 128], ident)
                nc.any.tensor_copy(Vt[:, tb * (TB // 128) + u, :], ps_t)

            # rms scale + rope per j-block. The ones128 matmul leaves
            # sum_d(x^2) replicated on every output row, so the whole scale
            # chain stays [128, TB] and no row-broadcast is needed.
            for j in range(QH + 1):
                sq = sqp.tile([128, TB], bf, tag="sq", name=f"sq_{tb}_{j}")
                nc.vector.tensor_tensor(sq, stage[:, j], stage[:, j], mult)
                ps_ss = psum.tile([128, TB], f32, name=f"psss_{tb}_{j}", tag="bank")
                nc.tensor.matmul(ps_ss, lhsT=ones128, rhs=sq, start=True, stop=True)
                rms = rmsp.tile([128, TB], f32, tag="rms", name=f"rms_{tb}_{j}")
                nc.scalar.activation(rms, ps_ss, Sqrt, bias=eps_sb, scale=1.0 / D)
                scl = rmsp.tile([128, TB], f32, tag="scl", name=f"scl_{tb}_{j}")
                nc.vector.reciprocal_approx_fast(scl, rms)

                cw = cwq_sb if j < QH else cwk_sb
                sw = swq_sb if j < QH else swk_sb
                # rope on the UNnormalized row (rotation commutes with the
                # per-column scale); rotate-half via output-partition-shifted
                # DVE ops against the host-rotated sin table.
                a_t = ropep.tile([128, TB], f32, tag="a_t", name=f"at_{tb}_{j}")
                nc.vector.tensor_tensor(a_t, stage[:, j], cw[:, ts_], mult)
                b_t = ropep.tile([128, TB], f32, tag="b_t", name=f"bt_{tb}_{j}")
                nc.vector.tensor_tensor(
                    b_t[0:64, :], stage[64:128, j], sw[64:128, ts_], mult
                )
                nc.vector.tensor_tensor(
                    b_t[64:128, :], stage[0:64, j], sw[0:64, ts_], mult
                )
                nc.vector.tensor_tensor(a_t, a_t, b_t, add)
                dest = qTt[:, j] if j < QH else kT[:, ts_]
                nc.vector.tensor_tensor(dest, a_t, scl, mult)

        attnTs = {}

        def phase_b(tb):
            """attention for t block tb (attnT kept for phase_c)."""
            t0 = tb * TB
            # o = sb - 4*tb; o=0 (full col range) goes FIRST so the
            # start=True PV/rowsum matmuls cover the whole bank; later
            # partial-range matmuls accumulate onto uniformly-written bytes
            # (CoreSim requires this; matches HW has_written semantics).
            obs = [0] + [o for o in range(-8, 4) if o != 0 and 4 * tb + o >= 0]

            def cr(o):
                if o >= 0:
                    return 128 * o, TB
                elif o >= -4:
                    return 0, TB
                else:
                    return 0, 128 * (o + 9)

            attnT = attnp.tile([128, QH, TB], bf, tag="attnT", name=f"attnT_{tb}")
            attnTs[tb] = attnT
            for hp in range(QH // 2):
                heads = (2 * hp, 2 * hp + 1)
                es = {}
                nobs = len(obs)
                rs_ps = {
                    h: psum.tile([128, TB], f32, name=f"psr_{tb}_{h}", tag="bank")
                    for h in heads
                }

                def rs_mm(h, oi):
                    # rowsum via ones128: result replicated on all rows, so
                    # the reciprocal is directly usable as a [128, TB] factor.
                    c0, c1 = cr(obs[oi])
                    nc.tensor.matmul(
                        rs_ps[h][:, c0:c1],
                        lhsT=ones128,
                        rhs=es[h, oi][:, c0:c1],
                        start=(oi == 0),
                        stop=(oi == nobs - 1),
                        skip_group_check=True,
                    )

                # score + exp pass, with the rowsum matmuls chasing the exp
                # stream at a fixed lag so they never queue up behind a full
                # head's worth of ACT latency.
                LAG = 3
                for oi, o in enumerate(obs):
                    sb = 4 * tb + o
                    c0, c1 = cr(o)
                    for h in heads:
                        ps_s = psum.tile(
                            [128, TB], f32, name=f"pss_{tb}_{h}_{oi}", tag="bank"
                        )
                        nc.tensor.matmul(
                            ps_s[:, c0:c1],
                            lhsT=kT[:, sb * 128 : (sb + 1) * 128],
                            rhs=qTs[tb][:, h, c0:c1],
                            start=True,
                            stop=True,
                        )
                        if o >= 0:  # causal strip at cols [128o, 128o+128)
                            u0 = 128 * o
                            nc.vector.tensor_tensor(
                                ps_s[:, u0 : u0 + 128], ps_s[:, u0 : u0 + 128],
                                maskd_sb, add,
                            )
                        elif o <= -5:  # window strip
                            u0 = 128 * (o + 8)
                            nc.vector.tensor_tensor(
                                ps_s[:, u0 : u0 + 128], ps_s[:, u0 : u0 + 128],
                                maskw_sb, add,
                            )
                        e = esp.tile([128, TB], bf, tag="es", name=f"es_{tb}_{h}_{oi}")
                        nc.scalar.activation(e[:, c0:c1], ps_s[:, c0:c1], Exp)
                        es[h, oi] = e
                    if oi >= LAG:
                        for h in heads:
                            rs_mm(h, oi - LAG)
                for oi in range(max(0, nobs - LAG), nobs):
                    for h in heads:
                        rs_mm(h, oi)
                for h in heads:
                    rb = rbp.tile([128, TB], f32, tag="rb", name=f"rb_{tb}_{h}")
                    nc.vector.reciprocal_approx_fast(rb, rs_ps[h])
                    pv = psum.tile([128, TB], f32, name=f"pspv_{tb}_{h}", tag="bank")
                    for oi, o in enumerate(obs):
                        sb = 4 * tb + o
                        c0, c1 = cr(o)
                        nc.tensor.matmul(
                            pv[:, c0:c1],
                            lhsT=Vt[:, sb, :],
                            rhs=es[h, oi][:, c0:c1],
                            start=(oi == 0),
                            stop=(oi == nobs - 1),
                            skip_group_check=True,
                        )
                    nc.vector.tensor_tensor(attnT[:, h], pv, rb, mult)

        def phase_c(tb):
            """o_proj partial for t block tb (store ob quads as one DMA)."""
            attnT = attnTs.pop(tb)
            for obp in range(NOB // 4):
                o_st = outp.tile(
                    [128, 4, TB], bf, tag="o_st", name=f"ost_{tb}_{obp}"
                )
                for oi in range(4):
                    ob = 4 * obp + oi
                    ps_o = psum.tile([128, TB], f32, name=f"pso_{tb}_{ob}", tag="bank")
                    for jc in range(QH):
                        nc.tensor.matmul(
                            ps_o,
                            lhsT=wo_sb[jc][:, ob * 128 : (ob + 1) * 128],
                            rhs=attnT[:, jc, :],
                            start=(jc == 0),
                            stop=(jc == QH - 1),
                        )
                    nc.vector.tensor_copy(o_st[:, oi, :], ps_o)
                st_eng = nc.gpsimd
                if tb == NT - 1 and obp % 2 == 1:
                    st_eng = nc.scalar
                st_eng.dma_start(
                    outr[tb * (NOB // 4) + obp].rearrange("p (oi u) -> p oi u", u=TB),
                    o_st,
                )

        # Software pipeline: phase A one t-block ahead (next block's qkv runs
        # while this block's rope/softmax chains sit on DVE/ACT), phase C one
        # block behind (o_proj matmuls fill any attention-phase PE bubbles).
        phase_a(0)
        # wo rides the gpsimd queue behind the startup w chunks/tables; the
        # gpsimd engine is idle then, so the issues aren't trapped behind
        # paced x-tile slot waits (sync) or the ACT exp bursts (scalar).
        for jc in range(QH):
            (nc.gpsimd if jc < 2 else nc.scalar).dma_start(wo_sb[jc], wor[jc])
        phase_a(1)
        phase_b(0)
        # per iteration: C first (dense, dependency-free filler), then B so
        # its score matmuls feed the ACT exp stream before the PE runs dry,
        # then A as the low-priority filler for B's rowsum/PV exp-chasing.
        for tb in range(1, NT):
            phase_c(tb - 1)
            phase_b(tb)
            if tb + 1 < NT:
                phase_a(tb + 1)
        phase_c(NT - 1)

    nc.compile()
    return nc


def _get_program(T):
    if T not in _PROG_CACHE:
        _PROG_CACHE[T] = _build_program(T)
    return _PROG_CACHE[T]


def _host_prep(positions, hidden_states, wqkv, wo, q_norm_w, k_norm_w):
    """Build the 8 per-core input maps (host-side sharding + table prep)."""
    T = hidden_states.shape[0]
    pos = np.asarray(positions).astype(np.float64)
    hs = np.asarray(hidden_states, dtype=np.float32)
    wqkv = np.asarray(wqkv, dtype=np.float32)
    wo = np.asarray(wo, dtype=np.float32)
    qw = np.asarray(q_norm_w, dtype=np.float64)
    kw = np.asarray(k_norm_w, dtype=np.float64)

    half = D // 2
    inv_freq = 1.0 / (THETA ** (np.arange(0, D, 2, dtype=np.float64) / D))  # [64]
    th = pos[:, None] * inv_freq[None, :]          # [T, 64]
    cos = np.cos(th).T                             # [64, T] float64
    sin = np.sin(th).T

    def tables(w, scale):
        cw = np.empty((D, T), np.float64)
        sw = np.empty((D, T), np.float64)
        cw[:half] = cos * (w[:half, None] * scale)
        cw[half:] = cos * (w[half:, None] * scale)
        # sin table pre-rotated by half so the kernel's shifted-output DVE
        # ops read coefficient and source from the SAME partition range:
        #   out[0:64]  = x[64:128] * sw[64:128]  (= -sin * w_hi * x_hi)
        #   out[64:128] = x[0:64]  * sw[0:64]    (= +sin * w_lo * x_lo)
        sw[:half] = sin * (w[:half, None] * scale)
        sw[half:] = -sin * (w[half:, None] * scale)
        return cw.astype(BF16), sw.astype(BF16)

    cwq, swq = tables(qw, SCALE)
    cwk, swk = tables(kw, 1.0)

    si = np.arange(128)[:, None]
    ui = np.arange(128)[None, :]
    maskd = np.where(ui >= si, 0.0, NEG).astype(np.float32)
    maskw = np.where(ui < si, 0.0, NEG).astype(np.float32)

    # tiled layout: block (tb, cq) = [128, 4*TB]; row p holds c-chunks
    # 4cq..4cq+3 back to back (4KB contiguous per partition)
    NTb, NCq = T // TB, HID // 512
    xT = np.ascontiguousarray(
        hs.T.reshape(NCq, 4, 128, NTb, TB)
        .transpose(3, 0, 2, 1, 4)
        .reshape(NTb * NCq * 128, 4 * TB)
    ).astype(BF16)

    NWC = 16
    in_maps = []
    for m in range(M):
        wq_m = wqkv[m * QH * D : (m + 1) * QH * D]            # [512, HID]
        wk_m = wqkv[H * D + m * D : H * D + (m + 1) * D]      # [128, HID]
        wv_m = wqkv[(H + HK) * D + m * D : (H + HK) * D + (m + 1) * D]
        wqkvT_m = np.concatenate([wq_m, wk_m, wv_m], axis=0).T  # [HID, 768]
        # pre-tile: chunk k = c-chunks [2k, 2k+2) as [128, 2*768] with the
        # per-partition row contiguous (c-major within the row)
        wqkvT_m = np.ascontiguousarray(
            wqkvT_m.reshape(NWC, 2, 128, NJ * D)
            .transpose(0, 2, 1, 3)
            .reshape(NWC * 128, 2 * NJ * D)
        ).astype(BF16)
        # pre-tile wo: chunk jc = [128, HID] contiguous
        woT_m = np.ascontiguousarray(
            wo[:, m * QH * D : (m + 1) * QH * D].T.reshape(QH * 128, HID)
        ).astype(BF16)                                        # [512, HID]
        in_maps.append(
            {
                "xT": xT,
                "wqkvT": wqkvT_m,
                "woT": woT_m,
                "cwq": cwq,
                "swq": swq,
                "cwk": cwk,
                "swk": swk,
                "maskd": maskd,
                "maskw": maskw,
            }
        )
    return in_maps


def _run(in_maps, T, trace=False):
    from concourse import bass_utils

    nc = _get_program(T)
    res = bass_utils.run_bass_kernel_spmd(
        nc, in_maps, core_ids=list(range(M)), trace=trace
    )
    return res


def kernel(positions, hidden_states, wqkv, wo, q_norm_w, k_norm_w, _trace=False):
    T = hidden_states.shape[0]
    in_maps = _host_prep(positions, hidden_states, wqkv, wo, q_norm_w, k_norm_w)
    res = _run(in_maps, T, trace=_trace)
    NTb, NOBq = T // TB, HID // 512
    acc = np.zeros((NTb, NOBq, 128, 4, TB), np.float64)
    for r in res.results:
        acc += r["outT"].astype(np.float64).reshape(NTb, NOBq, 128, 4, TB)
    # untile: out[t, o] with o = (4*obq + oi)*128 + p, t = tb*TB + u
    out = np.ascontiguousarray(
        acc.transpose(0, 4, 1, 3, 2).reshape(T, HID)
    ).astype(np.float32)
    kernel._last_results = res
    return out
